# revision 8
# baseline (speedup 1.0000x reference)
"""Chamfer distance L2 (B=4, N=M=8192, D=3) on 8 TRN2 NeuronCores.

Sharding: core c handles batch b = c//2, xyz1-half h = c%2 (4096 query
points against all 8192 xyz2 points of the same batch).

Measured-engine-rate redesign of the V1 kernel (same math, fewer/wider
DVE ops — DVE is the bottleneck at ~0.52ns/elem + ~84ns/op):
  - PE: K=18 augmented bf16 matmul -> PSUM f32 [128x512] x16 per n-tile.
  - ScalarE: 4 ACTIVATE copies per n-tile, PSUM f32 -> SBUF fp16, into a
    single [128 x 8192] quad tile (1967ns each -> 252us total, engine 2).
  - VectorE rowmin (tree, all fp16 2x): t = min(q0,q1); t = min(t,q2);
    t = min(t,q3)  (3 ops FD=2048); finalize fold 1024 -> fold 512 ->
    reduce -> dist1[:,nt].
  - VectorE colacc: ONE in-place FD=8192 min per n-tile.
  - Tail: PE transposes colacc 128x128 blocks -> PSUM fp16; VectorE
    strided-reduce -> dist2 partials.
Host: means + min-combine of the two per-batch halves (O(N) work only).
"""

import sys

for _p in ("/opt/trn_rl_repo",):
    if _p not in sys.path:
        sys.path.insert(0, _p)

from contextlib import ExitStack

import numpy as np
import ml_dtypes

import concourse.bacc as bacc
import concourse.bass as bass
import concourse.mybir as mybir
import concourse.tile as tile
from concourse import masks
from concourse.bass_utils import run_bass_kernel_spmd

WEIGHT = 0.6
B = 4
N = 8192
M = 8192
D = 3
NCORES = 8
HALF = N // 2

P = 128
NT = HALF // P  # 32 n-tiles per core
CHUNK = 2048
MC = M // CHUNK  # 4 m-chunks
MM_FREE = 512
K = 18

F32 = mybir.dt.float32
BF16 = mybir.dt.bfloat16
FP16 = mybir.dt.float16
MIN = mybir.AluOpType.min
AX = mybir.AxisListType.X
BF = ml_dtypes.bfloat16

_cached = None


def _build():
    nc = bacc.Bacc(
        "TRN2",
        target_bir_lowering=False,
        debug=False,
        enable_asserts=False,
        num_devices=NCORES,
    )

    lhs_d = nc.dram_tensor("lhs", [K, HALF], BF16, kind="ExternalInput")
    rhs_d = nc.dram_tensor("rhs", [K, M], BF16, kind="ExternalInput")
    out1_d = nc.dram_tensor("out1", [P, NT], F32, kind="ExternalOutput")
    out2_d = nc.dram_tensor("out2", [P, M // P], F32, kind="ExternalOutput")

    with tile.TileContext(nc) as tc, ExitStack() as ctx:
        const = ctx.enter_context(tc.tile_pool(name="const", bufs=1))
        qpool = ctx.enter_context(tc.tile_pool(name="q", bufs=3))
        spool = ctx.enter_context(tc.tile_pool(name="s", bufs=3))
        psum = ctx.enter_context(tc.tile_pool(name="ps", bufs=2, space="PSUM"))

        lhs_sb = const.tile([K, HALF], BF16)
        rhs_sb = const.tile([K, M], BF16)
        ident = const.tile([P, P], FP16)
        colacc = const.tile([P, M], FP16)
        dist1 = const.tile([P, NT], F32)
        dist2 = const.tile([P, M // P], F32)

        nc.sync.dma_start(lhs_sb[:, 0:P], lhs_d[:, 0:P])
        for c in range(MC):
            nc.sync.dma_start(
                rhs_sb[:, c * CHUNK : (c + 1) * CHUNK],
                rhs_d[:, c * CHUNK : (c + 1) * CHUNK],
            )
        nc.sync.dma_start(lhs_sb[:, P:HALF], lhs_d[:, P:HALF])
        masks.make_identity(nc, ident[:])

        for nt in range(NT):
            lhsT = lhs_sb[:, nt * P : (nt + 1) * P]
            if nt == 0:
                q = colacc[:]
            else:
                qt = qpool.tile([P, M], FP16, tag="q")
                q = qt[:]
            for mc in range(MC):
                pt = psum.tile([P, CHUNK], F32, tag="ps")
                for j in range(CHUNK // MM_FREE):
                    m0 = mc * CHUNK + j * MM_FREE
                    nc.tensor.matmul(
                        pt[:, j * MM_FREE : (j + 1) * MM_FREE],
                        lhsT,
                        rhs_sb[:, m0 : m0 + MM_FREE],
                        start=True,
                        stop=True,
                    )
                # ScalarE: PSUM f32 -> SBUF fp16 quad slice
                nc.scalar.copy(q[:, mc * CHUNK : (mc + 1) * CHUNK], pt[:])
            # VectorE row-min tree (fp16 2x); fold into a per-8-tile slot
            t = spool.tile([P, CHUNK], FP16, tag="t")
            f = spool.tile([P, 1024], FP16, tag="f")
            if nt % 8 == 0:
                g8t = spool.tile([P, 8 * 512], FP16, tag="g8")
                g8 = g8t[:]
            nc.vector.tensor_tensor(t[:], q[:, 0:2048], q[:, 2048:4096], MIN)
            nc.vector.tensor_tensor(t[:], t[:], q[:, 4096:6144], MIN)
            nc.vector.tensor_tensor(t[:], t[:], q[:, 6144:8192], MIN)
            nc.vector.tensor_tensor(f[:], t[:, 0:1024], t[:, 1024:2048], MIN)
            nc.vector.tensor_tensor(
                g8[:, (nt % 8) * 512 : (nt % 8) * 512 + 512],
                f[:, 0:512],
                f[:, 512:1024],
                MIN,
            )
            if nt % 4 == 3:
                nc.vector.tensor_reduce(
                    dist1[:, nt - 3 : nt + 1],
                    g8[:, (nt % 8 - 3) * 512 : (nt % 8 + 1) * 512].rearrange(
                        "p (b x) -> p b x", x=512
                    ),
                    axis=AX,
                    op=MIN,
                )
            # VectorE colacc: one wide in-place min per n-tile
            if nt > 0:
                nc.vector.tensor_tensor(colacc[:], colacc[:], q[:], MIN)

        # dist2 tail: transpose colacc 128x128 blocks, reduce old partitions
        for g in range(M // P // 8):
            tp = psum.tile([P, 8 * P], FP16, tag="ps")
            for b in range(8):
                t_ = g * 8 + b
                nc.tensor.transpose(
                    tp[:, b * P : (b + 1) * P],
                    colacc[:, t_ * P : (t_ + 1) * P],
                    ident[:],
                )
            nc.vector.tensor_reduce(
                dist2[:, g * 8 : (g + 1) * 8],
                tp[:].rearrange("p (b x) -> p b x", x=P),
                axis=AX,
                op=MIN,
            )

        nc.sync.dma_start(out1_d[:], dist1[:])
        nc.sync.dma_start(out2_d[:], dist2[:])

    nc.compile()
    return nc


def _get_nc():
    global _cached
    if _cached is None:
        _cached = _build()
    return _cached


def _split3(v):
    h = v.astype(BF)
    r = v - h.astype(np.float64)
    m = r.astype(BF)
    l = (r - m.astype(np.float64)).astype(BF)
    return h, m, l


def _in_maps(xyz1, xyz2):
    xyz1 = np.ascontiguousarray(np.asarray(xyz1, dtype=np.float32))
    xyz2 = np.ascontiguousarray(np.asarray(xyz2, dtype=np.float32))
    maps = []
    for c in range(NCORES):
        b, h = divmod(c, 2)
        X = xyz1[b, h * HALF : (h + 1) * HALF].astype(np.float64)
        Y = xyz2[b].astype(np.float64)

        xh = X.astype(BF)
        xl = (X - xh.astype(np.float64)).astype(BF)
        yh = Y.astype(BF)
        yl = (Y - yh.astype(np.float64)).astype(BF)
        Xr = xh.astype(np.float64) + xl.astype(np.float64)
        Yr = yh.astype(np.float64) + yl.astype(np.float64)
        s1h, s1m, s1l = _split3(np.einsum("nd,nd->n", Xr, Xr))
        s2h, s2m, s2l = _split3(np.einsum("md,md->m", Yr, Yr))

        lhs = np.empty((K, HALF), BF)
        lhs[0:3] = 1.0
        lhs[3] = s1h
        lhs[4] = s1m
        lhs[5] = s1l
        lhs[6:9] = (-2.0 * xh.astype(np.float64)).astype(BF).T
        lhs[9:12] = lhs[6:9]
        lhs[12:15] = (-2.0 * xl.astype(np.float64)).astype(BF).T
        lhs[15:18] = lhs[12:15]

        rhs = np.empty((K, M), BF)
        rhs[0] = s2h
        rhs[1] = s2m
        rhs[2] = s2l
        rhs[3:6] = 1.0
        rhs[6:9] = yh.T
        rhs[9:12] = yl.T
        rhs[12:15] = yh.T
        rhs[15:18] = yl.T
        maps.append({"lhs": lhs, "rhs": rhs})
    return maps


def _combine(results):
    d1 = np.concatenate([results[c]["out1"].T.reshape(-1) for c in range(NCORES)])
    d2 = np.concatenate(
        [
            np.minimum(results[2 * b]["out2"], results[2 * b + 1]["out2"]).T.reshape(-1)
            for b in range(B)
        ]
    )
    val = WEIGHT * (np.float64(d1.mean()) + np.float64(d2.mean())) / 2.0
    return np.float32(val)


def run(xyz1, xyz2, trace=False, **spmd_kwargs):
    nc = _get_nc()
    br = run_bass_kernel_spmd(
        nc, _in_maps(xyz1, xyz2), list(range(NCORES)), trace=trace, **spmd_kwargs
    )
    return _combine(br.results), br


def kernel(xyz1, xyz2):
    out, _ = run(xyz1, xyz2)
    return out


if __name__ == "__main__":
    rng = np.random.default_rng(0)
    a = rng.standard_normal((B, N, D)).astype(np.float32)
    b = rng.standard_normal((B, M, D)).astype(np.float32)
    print(kernel(a, b))


# revision 9
# speedup vs baseline: 1.0022x; 1.0022x over previous
"""Chamfer distance L2 (B=4, N=M=8192, D=3) on 8 TRN2 NeuronCores.

Sharding: core c handles batch b = c//2, xyz1-half h = c%2 (4096 query
points against all 8192 xyz2 points of the same batch).

Measured-engine-rate redesign of the V1 kernel (same math, fewer/wider
DVE ops — DVE is the bottleneck at ~0.52ns/elem + ~84ns/op):
  - PE: K=18 augmented bf16 matmul -> PSUM f32 [128x512] x16 per n-tile.
  - ScalarE: 4 ACTIVATE copies per n-tile, PSUM f32 -> SBUF fp16, into a
    single [128 x 8192] quad tile (1967ns each -> 252us total, engine 2).
  - VectorE rowmin (tree, all fp16 2x): t = min(q0,q1); t = min(t,q2);
    t = min(t,q3)  (3 ops FD=2048); finalize fold 1024 -> fold 512 ->
    reduce -> dist1[:,nt].
  - VectorE colacc: ONE in-place FD=8192 min per n-tile.
  - Tail: PE transposes colacc 128x128 blocks -> PSUM fp16; VectorE
    strided-reduce -> dist2 partials.
Host: means + min-combine of the two per-batch halves (O(N) work only).
"""

import sys

for _p in ("/opt/trn_rl_repo",):
    if _p not in sys.path:
        sys.path.insert(0, _p)

from contextlib import ExitStack

import numpy as np
import ml_dtypes

import concourse.bacc as bacc
import concourse.bass as bass
import concourse.mybir as mybir
import concourse.tile as tile
from concourse import masks
from concourse.bass_utils import run_bass_kernel_spmd

WEIGHT = 0.6
B = 4
N = 8192
M = 8192
D = 3
NCORES = 8
HALF = N // 2

P = 128
NT = HALF // P  # 32 n-tiles per core
CHUNK = 2048
MC = M // CHUNK  # 4 m-chunks
MM_FREE = 512
K = 18

F32 = mybir.dt.float32
BF16 = mybir.dt.bfloat16
FP16 = mybir.dt.float16
MIN = mybir.AluOpType.min
AX = mybir.AxisListType.X
BF = ml_dtypes.bfloat16

_cached = None


def _build():
    nc = bacc.Bacc(
        "TRN2",
        target_bir_lowering=False,
        debug=False,
        enable_asserts=False,
        num_devices=NCORES,
    )

    lhs_d = nc.dram_tensor("lhs", [K, HALF], BF16, kind="ExternalInput")
    rhs_d = nc.dram_tensor("rhs", [K, M], BF16, kind="ExternalInput")
    out1_d = nc.dram_tensor("out1", [P, NT], F32, kind="ExternalOutput")
    out2_d = nc.dram_tensor("out2", [P, M // P], F32, kind="ExternalOutput")

    with tile.TileContext(nc) as tc, ExitStack() as ctx:
        const = ctx.enter_context(tc.tile_pool(name="const", bufs=1))
        qpool = ctx.enter_context(tc.tile_pool(name="q", bufs=3))
        spool = ctx.enter_context(tc.tile_pool(name="s", bufs=2))
        psum = ctx.enter_context(tc.tile_pool(name="ps", bufs=2, space="PSUM"))

        lhs_sb = const.tile([K, HALF], BF16)
        rhs_sb = const.tile([K, M], BF16)
        ident = const.tile([P, P], FP16)
        colacc = const.tile([P, M], FP16)
        dist1 = const.tile([P, NT], F32)
        dist2 = const.tile([P, M // P], F32)

        nc.sync.dma_start(lhs_sb[:, 0:P], lhs_d[:, 0:P])
        for c in range(MC):
            nc.sync.dma_start(
                rhs_sb[:, c * CHUNK : (c + 1) * CHUNK],
                rhs_d[:, c * CHUNK : (c + 1) * CHUNK],
            )
        nc.sync.dma_start(lhs_sb[:, P:HALF], lhs_d[:, P:HALF])
        masks.make_identity(nc, ident[:])

        for nt in range(NT):
            lhsT = lhs_sb[:, nt * P : (nt + 1) * P]
            if nt == 0:
                q = colacc[:]
            else:
                qt = qpool.tile([P, M], FP16, tag="q")
                q = qt[:]
            for mc in range(MC):
                pt = psum.tile([P, CHUNK], F32, tag="ps")
                for j in range(CHUNK // MM_FREE):
                    m0 = mc * CHUNK + j * MM_FREE
                    nc.tensor.matmul(
                        pt[:, j * MM_FREE : (j + 1) * MM_FREE],
                        lhsT,
                        rhs_sb[:, m0 : m0 + MM_FREE],
                        start=True,
                        stop=True,
                    )
                # ScalarE: PSUM f32 -> SBUF fp16 quad slice
                nc.scalar.copy(q[:, mc * CHUNK : (mc + 1) * CHUNK], pt[:])
            # VectorE row-min tree (fp16 2x); fold into a per-8-tile slot
            t = spool.tile([P, CHUNK], FP16, tag="t")
            f = spool.tile([P, 1024], FP16, tag="f")
            if nt % 8 == 0:
                g8t = spool.tile([P, 8 * 512], FP16, tag="g8")
                g8 = g8t[:]
            nc.vector.tensor_tensor(t[:], q[:, 0:2048], q[:, 2048:4096], MIN)
            nc.vector.tensor_tensor(t[:], t[:], q[:, 4096:6144], MIN)
            nc.vector.tensor_tensor(t[:], t[:], q[:, 6144:8192], MIN)
            nc.vector.tensor_tensor(f[:], t[:, 0:1024], t[:, 1024:2048], MIN)
            nc.vector.tensor_tensor(
                g8[:, (nt % 8) * 512 : (nt % 8) * 512 + 512],
                f[:, 0:512],
                f[:, 512:1024],
                MIN,
            )
            if nt % 4 == 3:
                nc.vector.tensor_reduce(
                    dist1[:, nt - 3 : nt + 1],
                    g8[:, (nt % 8 - 3) * 512 : (nt % 8 + 1) * 512].rearrange(
                        "p (b x) -> p b x", x=512
                    ),
                    axis=AX,
                    op=MIN,
                )
            # VectorE colacc: one wide in-place min per n-tile
            if nt > 0:
                nc.vector.tensor_tensor(colacc[:], colacc[:], q[:], MIN)

        # dist2 tail: transpose colacc 128x128 blocks, reduce old partitions
        for g in range(M // P // 8):
            tp = psum.tile([P, 8 * P], FP16, tag="ps")
            for b in range(8):
                t_ = g * 8 + b
                nc.tensor.transpose(
                    tp[:, b * P : (b + 1) * P],
                    colacc[:, t_ * P : (t_ + 1) * P],
                    ident[:],
                )
            nc.vector.tensor_reduce(
                dist2[:, g * 8 : (g + 1) * 8],
                tp[:].rearrange("p (b x) -> p b x", x=P),
                axis=AX,
                op=MIN,
            )

        nc.sync.dma_start(out1_d[:], dist1[:])
        nc.sync.dma_start(out2_d[:], dist2[:])

    nc.compile()
    return nc


def _get_nc():
    global _cached
    if _cached is None:
        _cached = _build()
    return _cached


def _split3(v):
    h = v.astype(BF)
    r = v - h.astype(np.float64)
    m = r.astype(BF)
    l = (r - m.astype(np.float64)).astype(BF)
    return h, m, l


def _in_maps(xyz1, xyz2):
    xyz1 = np.ascontiguousarray(np.asarray(xyz1, dtype=np.float32))
    xyz2 = np.ascontiguousarray(np.asarray(xyz2, dtype=np.float32))
    maps = []
    for c in range(NCORES):
        b, h = divmod(c, 2)
        X = xyz1[b, h * HALF : (h + 1) * HALF].astype(np.float64)
        Y = xyz2[b].astype(np.float64)

        xh = X.astype(BF)
        xl = (X - xh.astype(np.float64)).astype(BF)
        yh = Y.astype(BF)
        yl = (Y - yh.astype(np.float64)).astype(BF)
        Xr = xh.astype(np.float64) + xl.astype(np.float64)
        Yr = yh.astype(np.float64) + yl.astype(np.float64)
        s1h, s1m, s1l = _split3(np.einsum("nd,nd->n", Xr, Xr))
        s2h, s2m, s2l = _split3(np.einsum("md,md->m", Yr, Yr))

        lhs = np.empty((K, HALF), BF)
        lhs[0:3] = 1.0
        lhs[3] = s1h
        lhs[4] = s1m
        lhs[5] = s1l
        lhs[6:9] = (-2.0 * xh.astype(np.float64)).astype(BF).T
        lhs[9:12] = lhs[6:9]
        lhs[12:15] = (-2.0 * xl.astype(np.float64)).astype(BF).T
        lhs[15:18] = lhs[12:15]

        rhs = np.empty((K, M), BF)
        rhs[0] = s2h
        rhs[1] = s2m
        rhs[2] = s2l
        rhs[3:6] = 1.0
        rhs[6:9] = yh.T
        rhs[9:12] = yl.T
        rhs[12:15] = yh.T
        rhs[15:18] = yl.T
        maps.append({"lhs": lhs, "rhs": rhs})
    return maps


def _combine(results):
    d1 = np.concatenate([results[c]["out1"].T.reshape(-1) for c in range(NCORES)])
    d2 = np.concatenate(
        [
            np.minimum(results[2 * b]["out2"], results[2 * b + 1]["out2"]).T.reshape(-1)
            for b in range(B)
        ]
    )
    val = WEIGHT * (np.float64(d1.mean()) + np.float64(d2.mean())) / 2.0
    return np.float32(val)


def run(xyz1, xyz2, trace=False, **spmd_kwargs):
    nc = _get_nc()
    br = run_bass_kernel_spmd(
        nc, _in_maps(xyz1, xyz2), list(range(NCORES)), trace=trace, **spmd_kwargs
    )
    return _combine(br.results), br


def kernel(xyz1, xyz2):
    out, _ = run(xyz1, xyz2)
    return out


if __name__ == "__main__":
    rng = np.random.default_rng(0)
    a = rng.standard_normal((B, N, D)).astype(np.float32)
    b = rng.standard_normal((B, M, D)).astype(np.float32)
    print(kernel(a, b))
